# revision 1
# baseline (speedup 1.0000x reference)
"""GCN layer (N=8192, Cin=Cout=32) on 8 Trainium2 NeuronCores.

Math (matches the PyG-style reference):
    A = 2*adj off-diagonal, 1 on the diagonal
    deg[j]   = sum_i A[i,j] = 2*(colsum_j(adj) - adj[j,j]) + 1
    dis      = deg ** -0.5
    y        = x @ W
    z        = dis[:,None] * y
    s[j,:]   = (A^T diag(dis) y)[j,:] = 2*(adj^T z)[j,:] + (1-2*adj[j,j])*z[j,:]
    out      = tanh(dis[:,None]*s + b).T          # [32, 8192]

Sharding: columns of adj (the target-node axis j) are split across the 8
cores; each core reads its 8192x1024 f32 slab from HBM exactly once
(32 MB/core = the memory roofline), converting to bf16 inline via SWDGE
cast-DMA and caching the whole slab in SBUF.  Column degrees are computed
with ones-matmuls on the PE while the slab streams in.  A tiny (4 KB/core)
AllGather shares dis across cores; phase 2 then runs entirely from SBUF:
s^T accumulates over 64 k-tiles as z_k^T @ slab_k on the PE, followed by the
diagonal fixup, dis[j] scaling, bias and tanh.  Each core writes its own
[32, 1024] column block of the output; the host concatenates.
"""

import numpy as np

import concourse.bass as bass
import concourse.bacc as bacc
import concourse.mybir as mybir
import concourse.tile as tile
from concourse import masks
from concourse.bass_utils import run_bass_kernel_spmd

F32 = mybir.dt.float32
BF16 = mybir.dt.bfloat16
AF = mybir.ActivationFunctionType
Alu = mybir.AluOpType

N = 8192          # nodes
C = 32            # channels (Cin == Cout)
NCORES = 8
JW = N // NCORES  # column-shard width per core (1024)
P = 128           # SBUF partitions
NT = N // P       # i-tiles (64)
TPC = 4           # i-tiles per slab DMA chunk
NCH = NT // TPC   # slab DMA chunks (16)
H = 512           # matmul free-dim chunk (one PSUM bank of f32)


def build_kernel(n_devices=NCORES, repeat=1, serialize=False, variant="full"):
    nc = bacc.Bacc(
        "TRN2", target_bir_lowering=False, debug=False, num_devices=n_devices
    )

    adjs = nc.dram_tensor("adjs", [N, JW], F32, kind="ExternalInput").ap()
    xT = nc.dram_tensor("xT", [C, N], F32, kind="ExternalInput").ap()
    xTo = nc.dram_tensor("xTo", [C, JW], F32, kind="ExternalInput").ap()
    w_d = nc.dram_tensor("W", [C, C], F32, kind="ExternalInput").ap()
    b_d = nc.dram_tensor("b", [C], F32, kind="ExternalInput").ap()
    adiag = nc.dram_tensor("adiag", [JW], F32, kind="ExternalInput").ap()
    out_d = nc.dram_tensor("out", [C, JW], F32, kind="ExternalOutput").ap()

    with tile.TileContext(nc) as tc:
        prev = None
        for _ in range(repeat):
            prev = _body(
                tc, adjs, xT, xTo, w_d, b_d, adiag, out_d, n_devices,
                prev_inst=prev if serialize else None,
                variant=variant,
            )

    nc.compile()
    return nc


def _body(tc, adjs, xT, xTo, w_d, b_d, adiag, out_d, n_devices=NCORES,
          prev_inst=None, variant="full"):
    nc = tc.nc

    # adj slab viewed as [128 partitions, 64 i-tiles, 1024 cols]
    adjs3 = adjs.rearrange("(t p) j -> p t j", p=P)

    with (
        tc.tile_pool(name="const", bufs=1) as constp,
        tc.tile_pool(name="slabp", bufs=1) as slabp,
        tc.tile_pool(name="zp", bufs=1) as zp,
        tc.tile_pool(name="rowsp", bufs=1) as rowsp,
        tc.tile_pool(name="bcp", bufs=1) as bcp,
        tc.tile_pool(name="outp", bufs=1) as outp,
        tc.tile_pool(name="xtp", bufs=2) as xtp,
        tc.tile_pool(name="psy", bufs=1, space="PSUM") as psy,
        tc.tile_pool(name="pss", bufs=1, space="PSUM") as pss,
        tc.tile_pool(name="pssm", bufs=1, space="PSUM") as pssm,
        tc.tile_pool(name="dramp", bufs=1, space="DRAM") as dramp,
    ):
        # ---- constants / small inputs ----
        ones32 = constp.tile([P, C], BF16)
        nc.vector.memset(ones32[:], 1.0)
        ones1q = constp.tile([P, C], F32)
        nc.vector.memset(ones1q[:], 1.0)
        ident64 = constp.tile([64, 64], F32)
        masks.make_identity(nc, ident64[:])
        w_sb = constp.tile([C, C], F32)
        nc.sync.dma_start(w_sb[:], w_d)
        # bias replicated into all four 32-partition quadrants
        b_quad = constp.tile([P, 1], F32)
        for g in range(4):
            nc.sync.dma_start(b_quad[g * C:(g + 1) * C, :], b_d)
        # adiag in quad-row layout: chunk g on partition 32g
        QW = JW // 4  # 256
        adiag_q = constp.tile([P, QW], F32)
        for g in range(4):
            nc.sync.dma_start(
                adiag_q[g * C:g * C + 1, :], adiag[g * QW:(g + 1) * QW]
            )
        xto_sb = constp.tile([C, JW], F32)
        nc.sync.dma_start(xto_sb[:], xTo)
        dis_col = constp.tile([P, NT], F32)

        slab = slabp.tile([P, NT, JW], BF16)      # 128 KB/partition
        z_sb = zp.tile([P, NT, C], BF16)

        # ---- y = x @ W, one [128, 32] block per i-tile, kept in PSUM ----
        y_ps = psy.tile([P, NT, C], F32)          # 4 PSUM banks
        XCH = 8                                   # i-tiles per xT chunk
        for ch in range(NT // XCH):
            xt_ch = xtp.tile([C, XCH * P], F32, tag="xt")
            nc.sync.dma_start(xt_ch[:], xT[:, ch * XCH * P:(ch + 1) * XCH * P])
            for tt in range(XCH):
                t = ch * XCH + tt
                nc.tensor.matmul(
                    y_ps[:, t, :],
                    xt_ch[:, tt * P:(tt + 1) * P],
                    w_sb[:],
                    start=True,
                    stop=True,
                )

        # ---- phase 1: stream slab in two column halves (f32 -> bf16 cast
        # DMAs).  Half h covers columns [512h, 512h+512) of every i-tile, so
        # the first half's column degrees — and its AllGather — complete
        # while the second half is still streaming. ----
        # asymmetric split: 768 cols (3 quads, 3KB DMA runs) then 256
        HS = [0, 3 * QW]          # half start cols
        HWD = [3 * QW, QW]        # half widths
        HG = [(0, 1, 2), (3,)]    # quad groups per half
        HJ = [(0, 6), (6, 2)]     # (j offset, j count) of i-tiles per core
        for h in range(2):
            for ch in range(NCH):
                d = nc.gpsimd.dma_start(
                    slab[:, ch * TPC:(ch + 1) * TPC, HS[h]:HS[h] + HWD[h]],
                    adjs3[:, ch * TPC:(ch + 1) * TPC, HS[h]:HS[h] + HWD[h]],
                )
                if prev_inst is not None and h == 0 and ch == 0:
                    # serialize-repeats bench mode: gate the first slab DMA
                    # of this pass on the previous pass's output DMA.  The
                    # Pool sequencer is in-order, so every later SWDGE DMA
                    # queues behind it — one cheap edge serializes the pass.
                    from concourse.tile_rust import add_dep_helper
                    add_dep_helper(
                        d.ins, prev_inst.ins, reason="serialize repeat"
                    )

        # colsums in quad layout: chunk g accumulates on psum partition 32g
        # via col-group tiling; half h owns groups {2h, 2h+1}.
        cs = pss.tile([P, QW], F32, tag="cs")     # 1 PSUM bank
        for h in range(2):
            for t in range(NT):
                for g in HG[h]:
                    nc.tensor.matmul(
                        cs[g * C:g * C + 1, :],
                        ones32[:, 0:1],
                        slab[:, t, g * QW:(g + 1) * QW],
                        start=(t == 0),
                        stop=(t == NT - 1),
                        tile_position=(0, g * C),
                    )

        # ---- per-half: degree rows, AllGather, dis transpose, z ----
        use_cc = n_devices > 1 and variant != "nocc"
        rdeg = rowsp.tile([P, QW], F32, tag="rdeg")
        dis_row = rowsp.tile([P, QW], F32, tag="dis")
        r3_row = rowsp.tile([P, QW], F32, tag="r3")
        cc_ins = [
            dramp.tile([HWD[h]], F32, name=f"cc_in{h}") for h in range(2)
        ]
        cc_outs = [
            dramp.tile(
                [NCORES * HWD[h]], F32,
                addr_space="Shared" if use_cc else "Local",
                name=f"cc_out{h}",
            )
            for h in range(2)
        ]

        for h in range(2):
            ph = slice(C * HG[h][0], C * (HG[h][-1] + 1))
            # deg = 2*(colsum - adiag) + 1; rdeg = 1/deg (in-place); only
            # partition rows {64h, 64h+32} carry real data.
            nc.vector.tensor_sub(rdeg[ph, :], cs[ph, :], adiag_q[ph, :])
            nc.vector.tensor_scalar(
                rdeg[ph, :], rdeg[ph, :], 2.0, 1.0, op0=Alu.mult, op1=Alu.add
            )
            nc.vector.reciprocal(rdeg[ph, :], rdeg[ph, :])
            nc.scalar.sqrt(dis_row[ph, :], rdeg[ph, :])
            nc.vector.tensor_scalar(
                r3_row[ph, :], adiag_q[ph, :], -1.0, 0.5,
                op0=Alu.mult, op1=Alu.add,
            )
            nc.vector.tensor_mul(r3_row[ph, :], r3_row[ph, :], rdeg[ph, :])
            # this half's 512 dis values -> DRAM (partitions {64h, 64h+32})
            nc.sync.dma_start(
                cc_ins[h][:], dis_row[C * HG[h][0]:C * (HG[h][-1] + 1):C, :]
            )

        if variant == "dmacs":
            return nc.sync.dma_start(out_d[:, 0:QW], dis_row[0:C, :])

        for h in range(2):
            if use_cc:
                nc.gpsimd.collective_compute(
                    "AllGather",
                    Alu.bypass,
                    replica_groups=[list(range(n_devices))],
                    ins=[cc_ins[h].opt()],
                    outs=[cc_outs[h].opt()],
                )
            else:
                for r in range(NCORES):
                    nc.sync.dma_start(
                        cc_outs[h][r * HWD[h]:(r + 1) * HWD[h]], cc_ins[h][:]
                    )

        # ---- per-half: dis transpose + z + phase-2 matmuls, in readiness
        # order (PE and DVE issue in-order, so nothing that waits on the
        # late AG_1 may be emitted before the dis-half-0 work). ----
        disTs = [
            bcp.tile([NCORES * HWD[h] // P, P], F32, name=f"disT{h}")
            for h in range(2)
        ]
        s_ps = pss.tile([P, QW], F32)
        tiles_h = [
            [c8 * 8 + HJ[h][0] + j for c8 in range(8) for j in range(HJ[h][1])]
            for h in range(2)
        ]
        first = {g: True for g in range(4)}
        for h in range(2):
            nc.sync.dma_start(
                disTs[h][:], cc_outs[h].rearrange("(t p) -> t p", p=P)
            )
            jo, jn = HJ[h]
            nk = NCORES * HWD[h] // P
            dis_ps = pssm.tile([P, nk], F32, tag="sm", name=f"dis_ps{h}")
            nc.tensor.transpose(dis_ps[:], disTs[h][:], ident64[0:nk, 0:nk])
            nc.vector.tensor_copy(
                dis_col.rearrange("p (c j) -> p c j", c=8)[:, :, jo:jo + jn],
                dis_ps.rearrange("p (c j) -> p c j", c=8),
            )
            # z for this half's i-tiles (8 strided blocks)
            for c8 in range(8):
                t0 = c8 * 8 + jo
                nc.vector.tensor_tensor(
                    z_sb[:, t0:t0 + jn, :],
                    y_ps[:, t0:t0 + jn, :],
                    dis_col[:, t0:t0 + jn].unsqueeze(2).broadcast_to(
                        [P, jn, C]
                    ),
                    op=Alu.mult,
                )
            if variant == "nophase2":
                continue
            # phase 2 for this half's i-tiles: quad col-tiled, 4 concurrent
            # M=32 matmuls per i-tile
            for t in tiles_h[h]:
                lastt = h == 1 and t == tiles_h[1][-1]
                for g in range(4):
                    nc.tensor.matmul(
                        s_ps[g * C:(g + 1) * C, :],
                        z_sb[:, t, :],
                        slab[:, t, g * QW:(g + 1) * QW],
                        start=first[g],
                        stop=lastt,
                        tile_position=(0, g * C),
                    )
                    first[g] = False

        # ---- quad broadcasts: quad rows -> [128, 256] (4 col groups) ----
        dis_bq = bcp.tile([P, QW], F32)
        r3_bq = bcp.tile([P, QW], F32)
        for row, dst in ((dis_row, dis_bq), (r3_row, r3_bq)):
            bc = pssm.tile([P, QW], F32, tag="sm")
            for g in range(4):
                nc.tensor.matmul(
                    bc[g * C:(g + 1) * C, :],
                    ones1q[g * C:g * C + 1, :],
                    row[g * C:g * C + 1, :],
                    start=True,
                    stop=True,
                    tile_position=(g * C, g * C),
                )
            nc.vector.tensor_copy(dst[:], bc[:])

        # ---- yT over own columns, quad layout [128, 256] ----
        yt_ps = pssm.tile([P, QW], F32, tag="sm")
        for g in range(4):
            nc.tensor.matmul(
                yt_ps[g * C:(g + 1) * C, :],
                w_sb[:],
                xto_sb[:, g * QW:(g + 1) * QW],
                start=True,
                stop=True,
                tile_position=(0, g * C),
            )

        if variant == "nophase2":
            return nc.sync.dma_start(out_d[:, 0:QW], dis_bq[0:C, :])

        # ---- epilogue: out = tanh(2*(dis*s + r3*yT) + b), quad layout ----
        u = outp.tile([P, QW], F32)
        v = outp.tile([P, QW], F32)
        nc.vector.tensor_mul(u[:], s_ps[:], dis_bq[:])
        nc.vector.tensor_mul(v[:], yt_ps[:], r3_bq[:])
        nc.vector.tensor_add(u[:], u[:], v[:])
        nc.scalar.activation(v[:], u[:], AF.Tanh, bias=b_quad[:], scale=2.0)
        last = None
        for g in range(4):
            last = nc.sync.dma_start(
                out_d[:, g * QW:(g + 1) * QW], v[g * C:(g + 1) * C, :]
            )
        return last


_NC_CACHE = None


def _get_nc():
    global _NC_CACHE
    if _NC_CACHE is None:
        _NC_CACHE = build_kernel()
    return _NC_CACHE


def kernel(x, adj, W, b, **run_kwargs):
    x = np.ascontiguousarray(np.asarray(x, dtype=np.float32))
    adj = np.ascontiguousarray(np.asarray(adj, dtype=np.float32))
    W = np.ascontiguousarray(np.asarray(W, dtype=np.float32))
    b = np.ascontiguousarray(np.asarray(b, dtype=np.float32))

    nc = _get_nc()
    xT = np.ascontiguousarray(x.T)
    diag = np.ascontiguousarray(np.diagonal(adj)).astype(np.float32)

    in_maps = []
    for c in range(NCORES):
        js = slice(c * JW, (c + 1) * JW)
        in_maps.append(
            {
                "adjs": np.ascontiguousarray(adj[:, js]),
                "xT": xT,
                "xTo": np.ascontiguousarray(xT[:, js]),
                "W": W,
                "b": b,
                "adiag": np.ascontiguousarray(diag[js]),
            }
        )

    res = run_bass_kernel_spmd(
        nc, in_maps, core_ids=list(range(NCORES)), **run_kwargs
    )
    out = np.concatenate(
        [res.results[c]["out"] for c in range(NCORES)], axis=1
    )
    if run_kwargs:
        return out, res
    return out

